# revision 1
# baseline (speedup 1.0000x reference)
"""KAN layer kernel for 8 Trainium2 NeuronCores.

Math (reference):
    basis[b,i] = sum_h silu(x[b,i]*w1[i%K,h] + b1[i%K,h]) * w2[i%K,h] + b2[i%K]
    out[b,o]   = sum_i basis[b,i] * Wsum[o,i],   Wsum = W.sum(-1)   # [O,I]

Sharding: data-parallel over the input-feature axis I (16384 -> 8 x 2048).
Each core computes a partial out[64,1024] over its feature slice; host sums.

Per-core device program (memory-bound on reading its 42 MB W slice):
  - W arrives host-transposed as Wt[i,k,o]; the k-reduction happens *inside
    the DMA* via serial accum_op=add transfers (SDMA CCE), so Wsum[i,o]
    lands in SBUF with zero engine work and contraction (i) already on the
    partition axis -- no on-chip transposes anywhere.
  - basis is computed with i on partitions: ACT evaluates
    silu(w1*x+b1) with per-partition scale/bias vectors; DVE accumulates
    w2*silu(+b2) via fused scalar_tensor_tensor. Result acc[i,b] is directly
    the matmul lhsT.
  - 2 fp32 matmuls per i-tile accumulate into PSUM across all 16 i-tiles.
"""
import numpy as np

B, I, O, K, H = 64, 16384, 1024, 5, 16
NCORES = 8
IC = I // NCORES          # 2048 features per core
P = 128                   # partition tile
NT = IC // P              # 16 i-tiles per core
NB = B                    # 64
NO = O                    # 1024
PRW = 3 * H + 1           # packed param cols per i-tile: w1,b1,w2 (16 ea) + b2
CBW = NT * NB + NT * PRW  # const tile width: x block + param block

TRACE = False             # test.py sets True to capture an NTFF profile
LAST_RESULT = None


def _build():
    from contextlib import ExitStack
    from concourse import bacc, mybir, tile

    dt = mybir.dt.float32
    nc = bacc.Bacc("TRN2", target_bir_lowering=False, debug=False,
                   num_devices=NCORES)
    Wt = nc.declare_dram_parameter("Wt", [IC, K, NO], dt, isOutput=False)
    cbd = nc.declare_dram_parameter("cb", [P, CBW], dt, isOutput=False)
    out = nc.declare_dram_parameter("out", [NB, NO], dt, isOutput=True)

    with tile.TileContext(nc) as tc, ExitStack() as ctx:
        const = ctx.enter_context(tc.tile_pool(name="const", bufs=1))
        wpool = ctx.enter_context(tc.tile_pool(name="w", bufs=8))
        bpool = ctx.enter_context(tc.tile_pool(name="basis", bufs=16))
        spool = ctx.enter_context(tc.tile_pool(name="silu", bufs=3))
        opool = ctx.enter_context(tc.tile_pool(name="out", bufs=1))
        psum = ctx.enter_context(tc.tile_pool(name="psum", bufs=1, space="PSUM"))

        cb = const.tile([P, CBW], dt)
        nc.sync.dma_start(cb[:, :], cbd[:, :])

        ps0 = psum.tile([NB, 512], dt, tag="ps0")
        ps1 = psum.tile([NB, 512], dt, tag="ps1")

        # ---- basisT[i,b] for every i-tile (ACT/DVE only; no W dependency) ----
        accs = []
        for t in range(NT):
            xs = cb[:, t * NB:(t + 1) * NB]
            pb = NT * NB + t * PRW
            acc = bpool.tile([P, NB], dt)
            for h in range(H):
                st = spool.tile([P, NB], dt)
                nc.scalar.activation(
                    st[:, :], xs, mybir.ActivationFunctionType.Silu,
                    bias=cb[:, pb + H + h:pb + H + h + 1],
                    scale=cb[:, pb + h:pb + h + 1],
                )
                if h == 0:
                    # acc = w2[:,0]*silu + b2
                    nc.vector.tensor_scalar(
                        acc[:, :], st[:, :],
                        cb[:, pb + 2 * H:pb + 2 * H + 1],
                        cb[:, pb + 3 * H:pb + 3 * H + 1],
                        op0=mybir.AluOpType.mult, op1=mybir.AluOpType.add,
                    )
                else:
                    # acc = w2[:,h]*silu + acc
                    nc.vector.scalar_tensor_tensor(
                        acc[:, :], st[:, :],
                        cb[:, pb + 2 * H + h:pb + 2 * H + h + 1],
                        acc[:, :],
                        op0=mybir.AluOpType.mult, op1=mybir.AluOpType.add,
                    )
            accs.append(acc)

        # ---- Wsum[i,o] = sum_k Wt[i,k,o], reduced inside the DMA.
        # All SWDGE DMAs issue in program order from the one gpsimd
        # sequencer, and step k of a tile must wait for step k-1's
        # completion (~2us). Interleaving the chains of a window of tiles
        # keeps every wait pre-satisfied so the queue never stalls. ----
        WIN = 4
        wsums = [None] * NT
        for base in range(0, NT, WIN):
            grp = range(base, min(base + WIN, NT))
            for t in grp:
                wsums[t] = wpool.tile([P, NO], dt, tag="wsum", name=f"wsum{t}")
            for k in range(K):
                for t in grp:
                    nc.gpsimd.dma_start(
                        wsums[t][:, :], Wt[t * P:(t + 1) * P, k, :],
                        accum_op=(mybir.AluOpType.bypass if k == 0
                                  else mybir.AluOpType.add))

        # ---- partial matmuls: out[b,o] += basisT.T @ Wsum ----
        for t in range(NT):
            nc.tensor.matmul(ps0[:, :], accs[t][:, :], wsums[t][:, 0:512],
                             start=(t == 0), stop=(t == NT - 1))
            nc.tensor.matmul(ps1[:, :], accs[t][:, :], wsums[t][:, 512:1024],
                             start=(t == 0), stop=(t == NT - 1))

        out_sb = opool.tile([NB, NO], dt)
        nc.vector.tensor_copy(out_sb[:, 0:512], ps0[:, :])
        nc.vector.tensor_copy(out_sb[:, 512:1024], ps1[:, :])
        nc.sync.dma_start(out[:, :], out_sb[:, :])
    nc.compile()
    return nc


def kernel(x, w1, b1, w2, b2, W):
    global LAST_RESULT
    from concourse.bass_utils import run_bass_kernel_spmd

    x = np.asarray(x, dtype=np.float32)
    W = np.asarray(W, dtype=np.float32)
    w1 = np.asarray(w1, dtype=np.float32)
    b1 = np.asarray(b1, dtype=np.float32)
    w2 = np.asarray(w2, dtype=np.float32)
    b2 = np.asarray(b2, dtype=np.float32)

    # ---- host prep: W -> [I,K,O] (contraction-major layout for the PE) ----
    Wt_full = np.ascontiguousarray(W.reshape(O, I * K).T).reshape(I, K, O)

    idx = np.arange(I) % K
    w1e, b1e, w2e = w1[idx], b1[idx], w2[idx]          # [I,H]
    b2e = b2[idx][:, None]                             # [I,1]
    pr = np.concatenate([w1e, b1e, w2e, b2e], axis=1)  # [I, PRW]

    in_maps = []
    for c in range(NCORES):
        sl = slice(c * IC, (c + 1) * IC)
        # x slice, transposed to [i, b], then swizzled to SBUF layout [P, NT*NB]
        xt = np.ascontiguousarray(x[:, sl].T)          # [IC, NB]
        xt_sb = xt.reshape(NT, P, NB).transpose(1, 0, 2).reshape(P, NT * NB)
        pr_sb = pr[sl].reshape(NT, P, PRW).transpose(1, 0, 2).reshape(P, NT * PRW)
        cb = np.ascontiguousarray(
            np.concatenate([xt_sb, pr_sb], axis=1), dtype=np.float32)
        in_maps.append({"Wt": Wt_full[sl], "cb": cb})

    nc = _build()
    res = run_bass_kernel_spmd(nc, in_maps, list(range(NCORES)), trace=TRACE)
    LAST_RESULT = res
    out = np.zeros((B, O), dtype=np.float32)
    for c in range(NCORES):
        out += res.results[c]["out"]
    return out



# revision 4
# speedup vs baseline: 2.6242x; 2.6242x over previous
"""KAN layer kernel for 8 Trainium2 NeuronCores.

Math (reference):
    basis[b,f] = sum_h silu(x[b,f]*w1[f%K,h] + b1[f%K,h]) * w2[f%K,h] + b2[f%K]
    out[b,o]   = sum_f basis[b,f] * Wsum[o,f],   Wsum = W.sum(-1)   # [O,F]

Sharding: features split 8 ways (each core holds ~2048 of the 16384
features and produces a partial out[64,1024]; host sums the partials).

Per-core device program (memory-bound on streaming its ~21 MB W slice):
  - W is cast to fp16 on the host (tolerance is 2e-2; fp16 keeps us ~1e-3)
    and laid out [k, tile, partition, o] so the k-reduction folds into the
    matmul contraction: no accum-DMA chains, no on-chip k-sum. The slice
    streams as 20 x 1 MB HWDGE DMAs at near HBM rate instead of the 80
    serialized 512 KB SWDGE CCE chains of the old version (~205 GB/s).
  - Features are permuted so each SBUF partition only holds features of a
    single f%K residue class. The silu affine params are then per-partition
    constants valid across every tile, so the basis needs just 16 wide ACT
    instructions (one per hidden unit, N=1088) + 16 wide DVE accumulates,
    instead of 256+256 narrow ones (ACT costs (N+352)/1.2 ns per instr --
    narrow instructions are overhead-dominated).
  - Slot grid is 17 deep per partition: 16 full [128 x 64b] basis tiles plus
    one 32-partition spill tile absorbing the residue-class remainders
    (class sizes aren't multiples of 16).
  - 2 PSUM banks accumulate out[64,1024] across all 85 contraction tiles.
"""
import numpy as np

B, I, O, K, H = 64, 16384, 1024, 5, 16
NCORES = 8
P = 128                   # SBUF partitions
NT = 16                   # full tiles (main slot grid depth)
T = NT + 1                # slots per partition incl. spill slot
M = 32                    # spill-tile partition count
TB = T * B                # basis free dim: 17*64 = 1088
TPC = 4                   # W tiles per DMA chunk (1 MB chunks)
NCHUNK = (K * NT) // TPC  # 20
CHW = TPC * O             # 4096 fp16 cols per chunk
CB = TB + 3 * H + 1       # const block: x_sb + w1e + b1e + w2e + b2e

TRACE = False             # test.py sets True to capture an NTFF profile
LAST_RESULT = None


def _plan_core(feats_by_class):
    """Assign one core's features to the (partition, slot) grid.

    Returns (cls_of_p[P], F17[P, T]) with F17 holding feature ids, -1 = pad.
    Every partition holds features of exactly one f%K class; spill slots
    (slot NT) only on partitions < M.
    """
    ks = [len(f) for f in feats_by_class]
    n = [-(-k // T) for k in ks]              # ceil(k/17) partitions minimum
    spare = P - sum(n)
    assert spare >= 0, (ks, n)
    for _ in range(spare):                     # kill the biggest spills first
        spills = [max(0, ks[c] - NT * n[c]) for c in range(K)]
        c = int(np.argmax(spills))
        n[c] += 1
    units = []                                 # (class, main[<=16], spill|-1)
    for c in range(K):
        fs = list(feats_by_class[c])
        main, sp = fs[: NT * n[c]], fs[NT * n[c]:]
        assert len(sp) <= n[c]
        for i in range(n[c]):
            units.append((c, main[NT * i: NT * (i + 1)],
                          sp[i] if i < len(sp) else -1))
    units.sort(key=lambda u: u[2] < 0)         # spill-carrying partitions first
    assert len(units) <= P
    n_spill = sum(1 for u in units if u[2] >= 0)
    assert n_spill <= M, n_spill
    units += [(0, [], -1)] * (P - len(units))
    cls_of_p = np.array([u[0] for u in units], dtype=np.int64)
    F17 = np.full((P, T), -1, dtype=np.int64)
    for p, (c, main, sp) in enumerate(units):
        F17[p, : len(main)] = main
        F17[p, NT] = sp
    return cls_of_p, F17


def _build():
    from contextlib import ExitStack
    from concourse import bacc, mybir, tile

    f32, f16 = mybir.dt.float32, mybir.dt.float16
    mult, add = mybir.AluOpType.mult, mybir.AluOpType.add
    nc = bacc.Bacc("TRN2", target_bir_lowering=False, debug=False,
                   num_devices=NCORES)
    Wm = nc.declare_dram_parameter("Wm", [NCHUNK, P, CHW], f16, isOutput=False)
    Wp = nc.declare_dram_parameter("Wp", [M, K * O], f16, isOutput=False)
    cbd = nc.declare_dram_parameter("cb", [P, CB], f32, isOutput=False)
    out = nc.declare_dram_parameter("out", [B, O], f32, isOutput=True)

    with tile.TileContext(nc) as tc, ExitStack() as ctx:
        const = ctx.enter_context(tc.tile_pool(name="const", bufs=1))
        wpool = ctx.enter_context(tc.tile_pool(name="w", bufs=12))
        wsp = ctx.enter_context(tc.tile_pool(name="wsp", bufs=1))
        spool = ctx.enter_context(tc.tile_pool(name="silu", bufs=6))
        apool = ctx.enter_context(tc.tile_pool(name="acc", bufs=1))
        opool = ctx.enter_context(tc.tile_pool(name="out", bufs=1))
        psum = ctx.enter_context(tc.tile_pool(name="psum", bufs=1, space="PSUM"))

        # const + spill-W ride the ACT HWDGE queue so the 20 x 1MB main-W
        # stream owns the SP HWDGE queue from t=0.
        cbt = const.tile([P, CB], f32)
        nc.scalar.dma_start(cbt[:, :], cbd[:, :])
        wpt = wsp.tile([M, K * O], f16)
        nc.scalar.dma_start(wpt[:, :], Wp[:, :])

        wtiles = []
        for c in range(NCHUNK):
            wt = wpool.tile([P, CHW], f16, tag="w", name=f"w{c}")
            nc.sync.dma_start(wt[:, :], Wm[c, :, :])
            wtiles.append(wt)

        # ---- basis: acc[p, t*B+b] = sum_h silu(x*w1+b1)*w2 + b2, fp16 ----
        xs = cbt[:, 0:TB]
        pw1, pb1, pw2, pb2 = TB, TB + H, TB + 2 * H, TB + 3 * H
        acc = apool.tile([P, TB], f16)
        for h in range(H):
            st = spool.tile([P, TB], f16, tag="st")
            nc.scalar.activation(
                st[:, :], xs, mybir.ActivationFunctionType.Silu,
                bias=cbt[:, pb1 + h:pb1 + h + 1],
                scale=cbt[:, pw1 + h:pw1 + h + 1],
            )
            if h == 0:
                nc.vector.tensor_scalar(
                    acc[:, :], st[:, :],
                    cbt[:, pw2:pw2 + 1], cbt[:, pb2:pb2 + 1],
                    op0=mult, op1=add)
            else:
                nc.vector.scalar_tensor_tensor(
                    acc[:, :], st[:, :], cbt[:, pw2 + h:pw2 + h + 1],
                    acc[:, :], op0=mult, op1=add)

        # ---- matmuls: out[b,o] += acc_tile.T @ W_tile over 85 tiles ----
        ps0 = psum.tile([B, 512], f32, tag="ps0")
        ps1 = psum.tile([B, 512], f32, tag="ps1")
        nmm = NCHUNK * TPC + K
        n = 0
        for c in range(NCHUNK):
            for g in range(TPC):
                t = (c * TPC + g) % NT
                lhsT = acc[:, t * B:(t + 1) * B]
                rhs = wtiles[c]
                nc.tensor.matmul(ps0[:, :], lhsT, rhs[:, g * O:g * O + 512],
                                 start=(n == 0), stop=(n == nmm - 1))
                nc.tensor.matmul(ps1[:, :], lhsT, rhs[:, g * O + 512:(g + 1) * O],
                                 start=(n == 0), stop=(n == nmm - 1))
                n += 1
        for k in range(K):
            lhsT = acc[0:M, NT * B:TB]
            nc.tensor.matmul(ps0[:, :], lhsT, wpt[0:M, k * O:k * O + 512],
                             start=False, stop=(n == nmm - 1))
            nc.tensor.matmul(ps1[:, :], lhsT, wpt[0:M, k * O + 512:(k + 1) * O],
                             start=False, stop=(n == nmm - 1))
            n += 1

        out_sb = opool.tile([B, O], f32)
        nc.vector.tensor_copy(out_sb[:, 0:512], ps0[:, :])
        nc.vector.tensor_copy(out_sb[:, 512:O], ps1[:, :])
        nc.sync.dma_start(out[:, :], out_sb[:, :])
    nc.compile()
    return nc


def kernel(x, w1, b1, w2, b2, W):
    global LAST_RESULT
    from concourse.bass_utils import run_bass_kernel_spmd

    x = np.asarray(x, dtype=np.float32)
    W = np.asarray(W, dtype=np.float32)
    w1 = np.asarray(w1, dtype=np.float32)
    b1 = np.asarray(b1, dtype=np.float32)
    w2 = np.asarray(w2, dtype=np.float32)
    b2 = np.asarray(b2, dtype=np.float32)

    # ---- host prep: W -> fp16 [f, k, o] with a zero row for pad slots ----
    Wt = np.zeros((I + 1, K, O), dtype=np.float16)
    Wt[:I] = W.transpose(1, 2, 0)
    xp = np.concatenate([x, np.zeros((B, 1), np.float32)], axis=1)

    in_maps = []
    seen = []
    for j in range(NCORES):
        feats = [np.arange(c, I, K)[j::NCORES] for c in range(K)]
        cls_of_p, F17 = _plan_core(feats)
        seen.append(F17[F17 >= 0].ravel())

        Fx = np.where(F17 < 0, I, F17)                 # pad -> zero col/row
        x_sb = xp[:, Fx].transpose(1, 2, 0).reshape(P, TB)
        pr = np.concatenate(
            [w1[cls_of_p], b1[cls_of_p], w2[cls_of_p], b2[cls_of_p][:, None]],
            axis=1)
        cb = np.ascontiguousarray(
            np.concatenate([x_sb, pr], axis=1), dtype=np.float32)

        A = Wt[Fx[:, :NT].T]                            # [t, p, k, o] fp16
        Wm = np.ascontiguousarray(
            A.transpose(2, 0, 1, 3)                     # [k, t, p, o]
            .reshape(NCHUNK, TPC, P, O)
            .transpose(0, 2, 1, 3)                      # [chunk, p, g, o]
            .reshape(NCHUNK, P, CHW))
        Wp_img = np.ascontiguousarray(Wt[Fx[:M, NT]].reshape(M, K * O))
        in_maps.append({"Wm": Wm, "Wp": Wp_img, "cb": cb})

    allf = np.sort(np.concatenate(seen))
    assert allf.shape == (I,) and np.array_equal(allf, np.arange(I))

    nc = _build()
    res = run_bass_kernel_spmd(nc, in_maps, list(range(NCORES)), trace=TRACE)
    LAST_RESULT = res
    out = np.zeros((B, O), dtype=np.float32)
    for c in range(NCORES):
        out += res.results[c]["out"]
    return out
